# revision 9
# baseline (speedup 1.0000x reference)
"""DepthWeightedAssignment Trainium2 kernel (v4: banded encode + fast head).

Per-detection argmin over 64 cameras of
  cost[i,j] = (d_i-c_j)^2 + 0.5*(1-exp(-0.045 c_j)) + 0.3*(t_i-t_j)^2/3600
sharded over 8 NeuronCores (N axis), threshold/weights on host.

Device algorithm (per core, N_C = 131072 detections):
  - One PE matmul per 512-column chunk computes the full encoded key
      X = 2^23 + 64*k + j,   k ~ round(768*cost)
    directly in PSUM.  The PE accumulates 32-row bands of the K axis in row
    order internally (verified empirically), so each detection gets its own
    32-row band holding its complete sequence
      [18 bf16-split data rows, A0..A2, +2^30 (quantizes S=49152*cost to the
       64*k grid at f32 ulp), -(2^30-2^23) (exact restore), +j]
    with the other detection's band zeroed on these stationary columns;
    cross-band combination adds 0, exact under any ordering.  K = 56:
    band L rows 0-23 (stationary cols 0-63 = cams), pad rows 24-31,
    band H rows 32-55 (cols 64-127).  PSUM: 128 partitions =
    (detL cams0-31 | detL cams32-63 | detH cams0-31 | detH cams32-63).
  - DVE tensor_reduce(min, axis=X, apply_transpose=True) reads PSUM directly
    and reduces each 32-camera partition group into the free dim; X is
    order-preserving in cost with j as tiebreak, exact where it matters.
  - Results stream to DRAM as a [128, 2048] f32 tile; host combines the two
    32-camera groups, unpacks j = X & 63, k = (X >> 6) - 2^17, applies the
    threshold and computes weights in float64.
Prep (device): bf16 triple splits of d-100, (d-100)^2, sw*(t-1800),
(sw*(t-1800))^2, staged through a DRAM scratch to relabel [128 part, F free]
tiles into [18, N_C] coefficient rows.  Detections map to prep-tile coords
via det = f*128 + p, so a free-dim chunk of 128 columns covers exactly the
16384 detections of one M-tile: chunk 0 runs on the otherwise-idle DVE to
minimize the serial head, later chunks run on Act+Pool hidden behind the
DVE reduce steady state.  Duplicated scratch rows are written with
broadcast-source DMAs (8 writes per half-chunk).
"""
import sys

sys.path.insert(0, "/opt/trn_rl_repo")

import numpy as np

N_TOTAL = 1 << 20
M_CAMS = 64
N_CORES = 8
N_C = N_TOTAL // N_CORES          # 131072 per core
HALF = N_C // 2                   # 65536 per prep half
CH = 8192                         # moving columns per M-tile
NQ = N_C // (2 * CH)              # 8 M-tiles per core (L+H windows per tile)
REG = 2048                        # psum region columns
NREG = CH // REG                  # 4 regions per M-tile
S_FINE = 768.0
SS = 64.0 * S_FINE                # 49152
H_BIAS = 2.0 ** 23
H_QUANT = 2.0 ** 30
H_REST = -(2.0 ** 30 - 2.0 ** 23)
W_T = 0.3 / 3600.0
DC = 100.0                        # depth centering
TCEN = 1800.0                     # time centering
THRESH_K = 625.0 * S_FINE         # 480000
K_BIAS = int(H_BIAS) >> 6         # 131072
KROWS = 56                        # K rows: band L 0-23, pad 24-31, band H 32-55

_CACHE = {}


def _build_module():
    import concourse.bacc as bacc
    import concourse.tile as tile
    from concourse import mybir

    f32 = mybir.dt.float32
    bf16 = mybir.dt.bfloat16
    AF = mybir.ActivationFunctionType
    OP = mybir.AluOpType
    AX = mybir.AxisListType

    nc = bacc.Bacc("TRN2", target_bir_lowering=False)

    ddep = nc.dram_tensor("ddep", [N_C], f32, kind="ExternalInput")
    dtim = nc.dram_tensor("dtim", [N_C], f32, kind="ExternalInput")
    stat_in = nc.dram_tensor("stat", [KROWS, 128], bf16, kind="ExternalInput")
    ones_in = nc.dram_tensor("ones14", [14, CH], bf16, kind="ExternalInput")
    x_out = nc.dram_tensor("xout", [128, 2048], f32, kind="ExternalOutput")
    # bf16 moving-row scratch, columns = core-local detection index
    # (det = hi*HALF + cw*16384 + p*128 + f for prep tile coords (p,
    # cw*128 + f), so a 128-col free chunk covers one M-tile's dets while
    # scratch writes stay 256B-contiguous per partition); 18 rows:
    #  0-2: d'_0 (coeffs B0 B1 B2) ; 3-4: d'_1 (B0 B1) ; 5: d'_2 (B0)
    #  6-8: q_0 q_1 q_2 (splits of d'^2, coeff SS)
    #  9-11: t_0 (C0 C1 C2) ; 12-13: t_1 (C0 C1) ; 14: t_2 (C0)
    #  15-17: u_0 u_1 u_2 (splits of tau^2, coeff SS)
    scratch = nc.dram_tensor("mscratch", [18, N_C], bf16)

    NCHUNK = 4
    CF = 128                      # free columns per prep chunk (16384 dets)

    with tile.TileContext(nc) as tc:
        with (
            tc.tile_pool(name="const", bufs=1) as cpool,
            tc.tile_pool(name="prep", bufs=3) as prep,
            tc.tile_pool(name="load", bufs=2) as load,
            tc.tile_pool(name="mov", bufs=3) as mpool,
            tc.tile_pool(name="ps", bufs=2, space="PSUM") as ppool,
            tc.tile_pool(name="s1", bufs=2) as s1pool,
        ):
            # ---- constants ----
            stat_t = cpool.tile([KROWS, 128], bf16)
            nc.sync.dma_start(stat_t[:], stat_in[:])
            ones_t = cpool.tile([14, CH], bf16)
            nc.scalar.dma_start(ones_t[:], ones_in[:])
            bias_d = cpool.tile([128, 1], f32)
            nc.gpsimd.memset(bias_d[:], -DC)
            sw = float(np.float32(np.sqrt(W_T)))
            bias_t = cpool.tile([128, 1], f32)
            nc.gpsimd.memset(bias_t[:], -sw * TCEN)
            scale_t = cpool.tile([128, 1], f32)
            nc.gpsimd.memset(scale_t[:], sw)
            one_t = cpool.tile([128, 1], f32)
            nc.gpsimd.memset(one_t[:], 1.0)

            loads = {}

            def do_load(hi):
                off = hi * HALF
                dload = load.tile([128, 512], f32, tag=f"dload{hi}")
                nc.sync.dma_start(
                    dload[:],
                    ddep[off:off + HALF].rearrange(
                        "(c p f) -> p c f", p=128, f=128),
                )
                tload = load.tile([128, 512], f32, tag=f"tload{hi}")
                nc.scalar.dma_start(
                    tload[:],
                    dtim[off:off + HALF].rearrange(
                        "(c p f) -> p c f", p=128, f=128),
                )
                loads[hi] = (dload, tload)

            # split-tile block -> scratch rows (per-row DMAs on the gpsimd
            # queue: Pool SEQ dispatch is ~25ns per dma_start)
            WR_PLAN = {
                "d1": ((0, 0), (1, 0), (2, 0), (3, 1), (4, 1), (5, 2)),
                "d2": ((6, 0), (7, 1), (8, 2)),
                "tau": ((9, 0), (10, 0), (11, 0), (12, 1), (13, 1), (14, 2)),
                "u": ((15, 0), (16, 1), (17, 2)),
            }

            def wr(bn, sp, hi, cw, F):
                c0 = hi * HALF + cw * F * 128
                for row, blk in WR_PLAN[bn]:
                    nc.gpsimd.dma_start(
                        scratch[row, c0:c0 + F * 128].rearrange(
                            "(c p f) -> p c f", p=128, f=128),
                        sp[:, blk * F:(blk + 1) * F],
                    )

            def do_prep_chunk(hi, cw, F, fast):
                dload, tload = loads[hi]
                fs = slice(cw * F, (cw + 1) * F)
                bases = (
                    ("d1", dload, AF.Identity, one_t, bias_d),
                    ("d2", dload, AF.Square, one_t, bias_d),
                    ("tau", tload, AF.Identity, scale_t, bias_t),
                    ("u", tload, AF.Square, scale_t, bias_t),
                )
                for bn, srct, fn, sc, bias in bases:
                    x = prep.tile([128, F], f32, tag=f"ppx{F}")
                    nc.scalar.activation(x[:], srct[:, fs], fn, bias=bias[:],
                                         scale=sc[:])
                    sp = prep.tile([128, 3 * F], bf16, tag=f"pps{F}")
                    r1 = prep.tile([128, F], f32, tag=f"ppr1{F}")
                    r2 = prep.tile([128, F], f32, tag=f"ppr2{F}")
                    s0, s1v, s2 = (sp[:, 0:F], sp[:, F:2 * F],
                                   sp[:, 2 * F:3 * F])
                    if fast:  # DVE chain (serial head, idle DVE)
                        nc.vector.tensor_copy(s0, x[:])
                        nc.vector.tensor_tensor(out=r1[:], in0=x[:], in1=s0,
                                                op=OP.subtract)
                        nc.vector.tensor_copy(s1v, r1[:])
                        nc.vector.tensor_tensor(out=r2[:], in0=r1[:], in1=s1v,
                                                op=OP.subtract)
                        nc.vector.tensor_copy(s2, r2[:])
                    else:     # Act casts + Pool subtracts (hidden)
                        nc.scalar.activation(s0, x[:], AF.Identity)
                        nc.gpsimd.tensor_tensor(out=r1[:], in0=x[:], in1=s0,
                                                op=OP.subtract)
                        nc.scalar.activation(s1v, r1[:], AF.Identity)
                        nc.gpsimd.tensor_tensor(out=r2[:], in0=r1[:], in1=s1v,
                                                op=OP.subtract)
                        nc.scalar.activation(s2, r2[:], AF.Identity)
                    wr(bn, sp, hi, cw, F)

            def do_mtile(q):
                m = mpool.tile([KROWS, CH], bf16, tag="m")
                nc.sync.dma_start(
                    m[0:18, :], scratch[:, 2 * q * CH:2 * q * CH + CH]
                )
                nc.sync.dma_start(
                    m[32:50, :], scratch[:, 2 * q * CH + CH:2 * (q + 1) * CH]
                )
                nc.scalar.dma_start(m[18:32, :], ones_t[:])
                nc.scalar.dma_start(m[50:56, :], ones_t[0:6, :])

                s1 = s1pool.tile([128, 256], f32, tag="s1")
                for r in range(NREG):
                    ps = ppool.tile([128, REG], f32, tag="ps")
                    for c in range(REG // 512):
                        col = r * REG + c * 512
                        nc.tensor.matmul(
                            ps[:, c * 512:(c + 1) * 512],
                            stat_t[:],
                            m[:, col:col + 512],
                            start=True,
                            stop=True,
                        )
                    nc.vector.tensor_reduce(
                        out=s1[:, r * 64:(r + 1) * 64],
                        in_=ps[:].rearrange("p (b j) -> p b j", j=32),
                        op=OP.min,
                        axis=AX.X,
                        apply_transpose=True,
                    )
                nc.scalar.dma_start(x_out[:, q * 256:(q + 1) * 256], s1[:])

            # ---- schedule ----
            do_load(0)
            do_prep_chunk(0, 0, CF, fast=True)    # head: DVE, covers q=0
            do_mtile(0)
            do_prep_chunk(0, 1, CF, fast=False)   # Pool/Act, covers q=1
            do_mtile(1)
            do_prep_chunk(0, 2, CF, fast=False)
            do_mtile(2)
            do_prep_chunk(0, 3, CF, fast=False)
            do_load(1)
            do_prep_chunk(1, 0, 512, fast=False)  # whole half 1 in one pass
            do_mtile(3)
            for q in range(4, NQ):
                do_mtile(q)

    nc.compile()
    return nc


def _host_consts(camera_depths, camera_times):
    import ml_dtypes
    bf = ml_dtypes.bfloat16

    def split3(x):
        x = np.asarray(x, np.float32)
        x0 = x.astype(bf).astype(np.float32)
        r1 = (x - x0).astype(np.float32)
        x1 = r1.astype(bf).astype(np.float32)
        r2 = (r1 - x1).astype(np.float32)
        x2 = r2.astype(bf).astype(np.float32)
        return x0, x1, x2

    cd = np.asarray(camera_depths, np.float64)
    ct = np.asarray(camera_times, np.float64)
    sw = float(np.float32(np.sqrt(W_T)))
    c1 = cd - DC
    t2c = sw * ct - sw * TCEN
    L = 0.5 * (1.0 - np.exp(-0.045 * cd))
    A = (SS * (c1 * c1 + t2c * t2c + L)).astype(np.float32)
    B = (SS * (-2.0 * c1)).astype(np.float32)
    C = (SS * (-2.0 * t2c)).astype(np.float32)
    Bs, Cs, As = split3(B), split3(C), split3(A)
    jall = np.arange(64, dtype=np.float32)

    stat = np.zeros((KROWS, 128), np.float32)
    # per-band rows (matches scratch layout):
    # r0+0..2: d0 x (B0 B1 B2) ; +3..4: d1 x (B0 B1) ; +5: d2 x B0
    # +6..8: SS ; +9..11: t0 x (C0 C1 C2) ; +12..13: t1 x (C0 C1) ;
    # +14: t2 x C0 ; +15..17: SS ; +18..20: A splits ; +21: +2^30 ;
    # +22: restore ; +23: j
    for r0, cols in ((0, slice(0, 64)), (32, slice(64, 128))):
        for ri, cf in enumerate([Bs[0], Bs[1], Bs[2], Bs[0], Bs[1], Bs[0]]):
            stat[r0 + ri, cols] = cf
        stat[r0 + 6:r0 + 9, cols] = SS
        for ri, cf in enumerate([Cs[0], Cs[1], Cs[2], Cs[0], Cs[1], Cs[0]]):
            stat[r0 + 9 + ri, cols] = cf
        stat[r0 + 15:r0 + 18, cols] = SS
        stat[r0 + 18, cols] = As[0]
        stat[r0 + 19, cols] = As[1]
        stat[r0 + 20, cols] = As[2]
        stat[r0 + 21, cols] = H_QUANT
        stat[r0 + 22, cols] = H_REST
        stat[r0 + 23, cols] = jall
    stat_b = stat.astype(bf)
    ones = np.ones((14, CH), bf)
    return stat_b, ones


def _det_perm():
    """(p', m) -> core-local det index, p' = 32g + j (g = cam group), m =
    256q + 64r + b; det = 16384q + 8192*(g>>1) + 2048r + 32b + j."""
    p = np.arange(128)[:, None]
    m = np.arange(2048)[None, :]
    g = p // 32
    j = p % 32
    q = m >> 8
    r = (m >> 6) & 3
    b = m & 63
    det = 16384 * q + 8192 * (g >> 1) + 2048 * r + 32 * b + j
    return det


def kernel(detection_depths, camera_depths, detection_times, camera_times):
    from concourse.bass_utils import run_bass_kernel_spmd

    if "nc" not in _CACHE:
        _CACHE["nc"] = _build_module()
        dp = _det_perm()
        _CACHE["perm"] = np.concatenate(
            [dp[0:32].ravel(), dp[64:96].ravel()])
    nc = _CACHE["nc"]
    perm = _CACHE["perm"]

    dd = np.ascontiguousarray(np.asarray(detection_depths, np.float32))
    dt = np.ascontiguousarray(np.asarray(detection_times, np.float32))
    stat, ones = _host_consts(camera_depths, camera_times)

    in_maps = []
    for c in range(N_CORES):
        sl = slice(c * N_C, (c + 1) * N_C)
        in_maps.append({
            "ddep": dd[sl].copy(),
            "dtim": dt[sl].copy(),
            "stat": stat,
            "ones14": ones,
        })
    results = run_bass_kernel_spmd(nc, in_maps, list(range(N_CORES))).results

    assignments = np.empty(N_TOTAL, np.int32)
    weights = np.empty(N_TOTAL, np.float32)
    for c in range(N_CORES):
        xo = results[c]["xout"]  # [128, 2048] f32
        xl = np.minimum(xo[0:32], xo[32:64])     # L dets, 64-cam min
        xh = np.minimum(xo[64:96], xo[96:128])   # H dets
        X = np.concatenate([xl.ravel(), xh.ravel()])
        ui = X.astype(np.int64)
        j = (ui & 63).astype(np.int32)
        kq = (ui >> 6) - K_BIAS
        np.maximum(kq, 0, out=kq)
        valid = kq < THRESH_K
        cost = kq.astype(np.float64) / S_FINE
        w = (1.0 / (1.0 + np.sqrt(cost))).astype(np.float32)
        base = c * N_C
        a_loc = np.empty(N_C, np.int32)
        w_loc = np.empty(N_C, np.float32)
        a_loc[perm] = np.where(valid, j, -1)
        w_loc[perm] = np.where(valid, w, np.float32(0.0))
        assignments[base:base + N_C] = a_loc
        weights[base:base + N_C] = w_loc
    return assignments, weights


# revision 11
# speedup vs baseline: 1.4085x; 1.4085x over previous
"""DepthWeightedAssignment Trainium2 kernel (v4: banded encode + fast head).

Per-detection argmin over 64 cameras of
  cost[i,j] = (d_i-c_j)^2 + 0.5*(1-exp(-0.045 c_j)) + 0.3*(t_i-t_j)^2/3600
sharded over 8 NeuronCores (N axis), threshold/weights on host.

Device algorithm (per core, N_C = 131072 detections):
  - One PE matmul per 512-column chunk computes the full encoded key
      X = 2^23 + 64*k + j,   k ~ round(768*cost)
    directly in PSUM.  The PE accumulates 32-row bands of the K axis in row
    order internally (verified empirically), so each detection gets its own
    32-row band holding its complete sequence
      [18 bf16-split data rows, A0..A2, +2^30 (quantizes S=49152*cost to the
       64*k grid at f32 ulp), -(2^30-2^23) (exact restore), +j]
    with the other detection's band zeroed on these stationary columns;
    cross-band combination adds 0, exact under any ordering.  K = 56:
    band L rows 0-23 (stationary cols 0-63 = cams), pad rows 24-31,
    band H rows 32-55 (cols 64-127).  PSUM: 128 partitions =
    (detL cams0-31 | detL cams32-63 | detH cams0-31 | detH cams32-63).
  - DVE tensor_reduce(min, axis=X, apply_transpose=True) reads PSUM directly
    and reduces each 32-camera partition group into the free dim; X is
    order-preserving in cost with j as tiebreak, exact where it matters.
  - Results stream to DRAM as a [128, 2048] f32 tile; host combines the two
    32-camera groups, unpacks j = X & 63, k = (X >> 6) - 2^17, applies the
    threshold and computes weights in float64.
Prep (device): bf16 triple splits of d-100, (d-100)^2, sw*(t-1800),
(sw*(t-1800))^2, staged through a DRAM scratch to relabel [128 part, F free]
tiles into [18, N_C] coefficient rows.  Detections map to prep-tile coords
via det = f*128 + p, so a free-dim chunk of 128 columns covers exactly the
16384 detections of one M-tile: chunk 0 runs on the otherwise-idle DVE to
minimize the serial head, later chunks run on Act+Pool hidden behind the
DVE reduce steady state.  Duplicated scratch rows are written with
broadcast-source DMAs (8 writes per half-chunk).
"""
import sys

sys.path.insert(0, "/opt/trn_rl_repo")

import numpy as np

N_TOTAL = 1 << 20
M_CAMS = 64
N_CORES = 8
N_C = N_TOTAL // N_CORES          # 131072 per core
HALF = N_C // 2                   # 65536 per prep half
CH = 8192                         # moving columns per M-tile
NQ = N_C // (2 * CH)              # 8 M-tiles per core (L+H windows per tile)
REG = 2048                        # psum region columns
NREG = CH // REG                  # 4 regions per M-tile
S_FINE = 768.0
SS = 64.0 * S_FINE                # 49152
H_BIAS = 2.0 ** 23
H_QUANT = 2.0 ** 30
H_REST = -(2.0 ** 30 - 2.0 ** 23)
W_T = 0.3 / 3600.0
DC = 100.0                        # depth centering
TCEN = 1800.0                     # time centering
THRESH_K = 625.0 * S_FINE         # 480000
K_BIAS = int(H_BIAS) >> 6         # 131072
KROWS = 56                        # K rows: band L 0-23, pad 24-31, band H 32-55

_CACHE = {}


def _build_module():
    import concourse.bacc as bacc
    import concourse.tile as tile
    from concourse import mybir

    f32 = mybir.dt.float32
    bf16 = mybir.dt.bfloat16
    AF = mybir.ActivationFunctionType
    OP = mybir.AluOpType
    AX = mybir.AxisListType

    nc = bacc.Bacc("TRN2", target_bir_lowering=False)

    ddep = nc.dram_tensor("ddep", [N_C], f32, kind="ExternalInput")
    dtim = nc.dram_tensor("dtim", [N_C], f32, kind="ExternalInput")
    stat_in = nc.dram_tensor("stat", [KROWS, 128], bf16, kind="ExternalInput")
    ones_in = nc.dram_tensor("ones14", [14, CH], bf16, kind="ExternalInput")
    x_out = nc.dram_tensor("xout", [128, 2048], f32, kind="ExternalOutput")
    # bf16 moving-row scratch, columns = core-local detection index
    # (det = hi*HALF + cw*16384 + p*128 + f for prep tile coords (p,
    # cw*128 + f), so a 128-col free chunk covers one M-tile's dets while
    # scratch writes stay 256B-contiguous per partition); 18 rows:
    #  0-2: d'_0 (coeffs B0 B1 B2) ; 3-4: d'_1 (B0 B1) ; 5: d'_2 (B0)
    #  6-8: q_0 q_1 q_2 (splits of d'^2, coeff SS)
    #  9-11: t_0 (C0 C1 C2) ; 12-13: t_1 (C0 C1) ; 14: t_2 (C0)
    #  15-17: u_0 u_1 u_2 (splits of tau^2, coeff SS)
    scratch = nc.dram_tensor("mscratch", [18, N_C], bf16)

    NCHUNK = 4
    CF = 128                      # free columns per prep chunk (16384 dets)

    with tile.TileContext(nc) as tc:
        with (
            tc.tile_pool(name="const", bufs=1) as cpool,
            tc.tile_pool(name="prep", bufs=3) as prep,
            tc.tile_pool(name="load", bufs=2) as load,
            tc.tile_pool(name="mov", bufs=3) as mpool,
            tc.tile_pool(name="ps", bufs=2, space="PSUM") as ppool,
            tc.tile_pool(name="s1", bufs=2) as s1pool,
        ):
            # ---- constants ----
            stat_t = cpool.tile([KROWS, 128], bf16)
            nc.sync.dma_start(stat_t[:], stat_in[:])
            ones_t = cpool.tile([14, CH], bf16)
            nc.scalar.dma_start(ones_t[:], ones_in[:])
            bias_d = cpool.tile([128, 1], f32)
            nc.gpsimd.memset(bias_d[:], -DC)
            sw = float(np.float32(np.sqrt(W_T)))
            bias_t = cpool.tile([128, 1], f32)
            nc.gpsimd.memset(bias_t[:], -sw * TCEN)
            scale_t = cpool.tile([128, 1], f32)
            nc.gpsimd.memset(scale_t[:], sw)
            one_t = cpool.tile([128, 1], f32)
            nc.gpsimd.memset(one_t[:], 1.0)

            loads = {}

            def do_load(hi):
                off = hi * HALF
                dload = load.tile([128, 512], f32, tag=f"dload{hi}")
                nc.sync.dma_start(
                    dload[:],
                    ddep[off:off + HALF].rearrange(
                        "(c p f) -> p c f", p=128, f=128),
                )
                tload = load.tile([128, 512], f32, tag=f"tload{hi}")
                nc.scalar.dma_start(
                    tload[:],
                    dtim[off:off + HALF].rearrange(
                        "(c p f) -> p c f", p=128, f=128),
                )
                loads[hi] = (dload, tload)

            # split-tile -> scratch rows: ONE DMA per base.  The dst row
            # dim sits inside the partition dim ("r (c p f) -> p r c f") so
            # it pairs with the SBUF free-dim block index; duplicate rows are
            # pre-duplicated as blocks in the split tile.
            WR_ROWS = {"d1": (0, 6), "d2": (6, 3), "tau": (9, 6),
                       "u": (15, 3)}

            def wr(bn, sp, hi, cw, F):
                c0 = hi * HALF + cw * F * 128
                row0, nr = WR_ROWS[bn]
                nc.sync.dma_start(
                    scratch[row0:row0 + nr, c0:c0 + F * 128].rearrange(
                        "r (c p f) -> p r c f", p=128, f=128),
                    sp[:, 0:nr * F].rearrange(
                        "p (b c f) -> p b c f", b=nr, f=128),
                )

            def do_prep_chunk(hi, cw, F, fast):
                dload, tload = loads[hi]
                fs = slice(cw * F, (cw + 1) * F)
                bases = (
                    ("d1", dload, AF.Identity, one_t, bias_d),
                    ("d2", dload, AF.Square, one_t, bias_d),
                    ("tau", tload, AF.Identity, scale_t, bias_t),
                    ("u", tload, AF.Square, scale_t, bias_t),
                )
                for bn, srct, fn, sc, bias in bases:
                    x = prep.tile([128, F], f32, tag=f"ppx{F}")
                    nc.scalar.activation(x[:], srct[:, fs], fn, bias=bias[:],
                                         scale=sc[:])
                    nb = WR_ROWS[bn][1]          # 6 (dup) or 3 blocks
                    sp = prep.tile([128, nb * F], bf16, tag=f"pps{nb}{F}")
                    r1 = prep.tile([128, F], f32, tag=f"ppr1{F}")
                    r2 = prep.tile([128, F], f32, tag=f"ppr2{F}")
                    if nb == 6:   # blocks [s0 s0 s0 s1 s1 s2]
                        s0d = (0, 1, 2)
                        s1d = (3, 4)
                        s2d = (5,)
                    else:         # blocks [s0 s1 s2]
                        s0d = (0,)
                        s1d = (1,)
                        s2d = (2,)
                    blk = lambda i: sp[:, i * F:(i + 1) * F]
                    if fast:  # DVE chain (serial head, idle DVE)
                        for i in s0d:
                            nc.vector.tensor_copy(blk(i), x[:])
                        nc.vector.tensor_tensor(out=r1[:], in0=x[:],
                                                in1=blk(s0d[0]),
                                                op=OP.subtract)
                        for i in s1d:
                            nc.vector.tensor_copy(blk(i), r1[:])
                        nc.vector.tensor_tensor(out=r2[:], in0=r1[:],
                                                in1=blk(s1d[0]),
                                                op=OP.subtract)
                        for i in s2d:
                            nc.vector.tensor_copy(blk(i), r2[:])
                    else:     # Act casts + Pool subtracts (hidden)
                        for i in s0d:
                            nc.scalar.activation(blk(i), x[:], AF.Identity)
                        nc.gpsimd.tensor_tensor(out=r1[:], in0=x[:],
                                                in1=blk(s0d[0]),
                                                op=OP.subtract)
                        for i in s1d:
                            nc.scalar.activation(blk(i), r1[:], AF.Identity)
                        nc.gpsimd.tensor_tensor(out=r2[:], in0=r1[:],
                                                in1=blk(s1d[0]),
                                                op=OP.subtract)
                        for i in s2d:
                            nc.scalar.activation(blk(i), r2[:], AF.Identity)
                    wr(bn, sp, hi, cw, F)

            def do_mtile(q):
                m = mpool.tile([KROWS, CH], bf16, tag="m")
                nc.sync.dma_start(
                    m[0:18, :], scratch[:, 2 * q * CH:2 * q * CH + CH]
                )
                nc.sync.dma_start(
                    m[32:50, :], scratch[:, 2 * q * CH + CH:2 * (q + 1) * CH]
                )
                nc.scalar.dma_start(m[18:32, :], ones_t[:])
                nc.scalar.dma_start(m[50:56, :], ones_t[0:6, :])

                s1 = s1pool.tile([128, 256], f32, tag="s1")
                for r in range(NREG):
                    ps = ppool.tile([128, REG], f32, tag="ps")
                    for c in range(REG // 512):
                        col = r * REG + c * 512
                        nc.tensor.matmul(
                            ps[:, c * 512:(c + 1) * 512],
                            stat_t[:],
                            m[:, col:col + 512],
                            start=True,
                            stop=True,
                        )
                    nc.vector.tensor_reduce(
                        out=s1[:, r * 64:(r + 1) * 64],
                        in_=ps[:].rearrange("p (b j) -> p b j", j=32),
                        op=OP.min,
                        axis=AX.X,
                        apply_transpose=True,
                    )
                nc.scalar.dma_start(x_out[:, q * 256:(q + 1) * 256], s1[:])

            # ---- schedule ----
            do_load(0)
            do_prep_chunk(0, 0, CF, fast=True)    # head: DVE, covers q=0
            do_mtile(0)
            do_prep_chunk(0, 1, CF, fast=False)   # Pool/Act, covers q=1
            do_mtile(1)
            do_prep_chunk(0, 2, CF, fast=False)
            do_mtile(2)
            do_prep_chunk(0, 3, CF, fast=False)
            do_load(1)
            for cw in range(NCHUNK):
                do_prep_chunk(1, cw, CF, fast=False)
            do_mtile(3)
            for q in range(4, NQ):
                do_mtile(q)

    nc.compile()
    return nc


def _host_consts(camera_depths, camera_times):
    import ml_dtypes
    bf = ml_dtypes.bfloat16

    def split3(x):
        x = np.asarray(x, np.float32)
        x0 = x.astype(bf).astype(np.float32)
        r1 = (x - x0).astype(np.float32)
        x1 = r1.astype(bf).astype(np.float32)
        r2 = (r1 - x1).astype(np.float32)
        x2 = r2.astype(bf).astype(np.float32)
        return x0, x1, x2

    cd = np.asarray(camera_depths, np.float64)
    ct = np.asarray(camera_times, np.float64)
    sw = float(np.float32(np.sqrt(W_T)))
    c1 = cd - DC
    t2c = sw * ct - sw * TCEN
    L = 0.5 * (1.0 - np.exp(-0.045 * cd))
    A = (SS * (c1 * c1 + t2c * t2c + L)).astype(np.float32)
    B = (SS * (-2.0 * c1)).astype(np.float32)
    C = (SS * (-2.0 * t2c)).astype(np.float32)
    Bs, Cs, As = split3(B), split3(C), split3(A)
    jall = np.arange(64, dtype=np.float32)

    stat = np.zeros((KROWS, 128), np.float32)
    # per-band rows (matches scratch layout):
    # r0+0..2: d0 x (B0 B1 B2) ; +3..4: d1 x (B0 B1) ; +5: d2 x B0
    # +6..8: SS ; +9..11: t0 x (C0 C1 C2) ; +12..13: t1 x (C0 C1) ;
    # +14: t2 x C0 ; +15..17: SS ; +18..20: A splits ; +21: +2^30 ;
    # +22: restore ; +23: j
    for r0, cols in ((0, slice(0, 64)), (32, slice(64, 128))):
        for ri, cf in enumerate([Bs[0], Bs[1], Bs[2], Bs[0], Bs[1], Bs[0]]):
            stat[r0 + ri, cols] = cf
        stat[r0 + 6:r0 + 9, cols] = SS
        for ri, cf in enumerate([Cs[0], Cs[1], Cs[2], Cs[0], Cs[1], Cs[0]]):
            stat[r0 + 9 + ri, cols] = cf
        stat[r0 + 15:r0 + 18, cols] = SS
        stat[r0 + 18, cols] = As[0]
        stat[r0 + 19, cols] = As[1]
        stat[r0 + 20, cols] = As[2]
        stat[r0 + 21, cols] = H_QUANT
        stat[r0 + 22, cols] = H_REST
        stat[r0 + 23, cols] = jall
    stat_b = stat.astype(bf)
    ones = np.ones((14, CH), bf)
    return stat_b, ones


def _det_perm():
    """(p', m) -> core-local det index, p' = 32g + j (g = cam group), m =
    256q + 64r + b; det = 16384q + 8192*(g>>1) + 2048r + 32b + j."""
    p = np.arange(128)[:, None]
    m = np.arange(2048)[None, :]
    g = p // 32
    j = p % 32
    q = m >> 8
    r = (m >> 6) & 3
    b = m & 63
    det = 16384 * q + 8192 * (g >> 1) + 2048 * r + 32 * b + j
    return det


def kernel(detection_depths, camera_depths, detection_times, camera_times):
    from concourse.bass_utils import run_bass_kernel_spmd

    if "nc" not in _CACHE:
        _CACHE["nc"] = _build_module()
        dp = _det_perm()
        _CACHE["perm"] = np.concatenate(
            [dp[0:32].ravel(), dp[64:96].ravel()])
    nc = _CACHE["nc"]
    perm = _CACHE["perm"]

    dd = np.ascontiguousarray(np.asarray(detection_depths, np.float32))
    dt = np.ascontiguousarray(np.asarray(detection_times, np.float32))
    stat, ones = _host_consts(camera_depths, camera_times)

    in_maps = []
    for c in range(N_CORES):
        sl = slice(c * N_C, (c + 1) * N_C)
        in_maps.append({
            "ddep": dd[sl].copy(),
            "dtim": dt[sl].copy(),
            "stat": stat,
            "ones14": ones,
        })
    results = run_bass_kernel_spmd(nc, in_maps, list(range(N_CORES))).results

    assignments = np.empty(N_TOTAL, np.int32)
    weights = np.empty(N_TOTAL, np.float32)
    for c in range(N_CORES):
        xo = results[c]["xout"]  # [128, 2048] f32
        xl = np.minimum(xo[0:32], xo[32:64])     # L dets, 64-cam min
        xh = np.minimum(xo[64:96], xo[96:128])   # H dets
        X = np.concatenate([xl.ravel(), xh.ravel()])
        ui = X.astype(np.int64)
        j = (ui & 63).astype(np.int32)
        kq = (ui >> 6) - K_BIAS
        np.maximum(kq, 0, out=kq)
        valid = kq < THRESH_K
        cost = kq.astype(np.float64) / S_FINE
        w = (1.0 / (1.0 + np.sqrt(cost))).astype(np.float32)
        base = c * N_C
        a_loc = np.empty(N_C, np.int32)
        w_loc = np.empty(N_C, np.float32)
        a_loc[perm] = np.where(valid, j, -1)
        w_loc[perm] = np.where(valid, w, np.float32(0.0))
        assignments[base:base + N_C] = a_loc
        weights[base:base + N_C] = w_loc
    return assignments, weights
